# revision 1
# baseline (speedup 1.0000x reference)
"""Trainium2 Bass kernel for the BaseMemory coref scoring module.

Computes, for full inputs (M=65536 memory slots, D=768, E=20, H=64):
    score = relu(pair @ W1 + b1) @ W2 + b2, masked with ent_counter>0,
    where pair = [mem, ment, mem*ment, dist_emb, cnt_emb].

Sharding: data-parallel over the cluster dimension M across 8 NeuronCores.
Each core's shard of mem_vectors is laid out [D, MS] (contraction-major) so
the PE consumes it directly; all FLOPs and all HBM traffic stay on device.

Key algebraic folds (host side, O(D*H) work):
  - mem@W1_mem + (mem*ment)@W1_had = mem @ (W1_mem + diag(ment)@W1_had)
  - ment@W1_ment + b1 folded into the 10-row dist bucket table
  - bucket embedding lookups become one-hot rows contracted on the PE
  - masking folded into the PE accumulation (exact)
"""

import os
import numpy as np

# The bass kernel executes through the axon PJRT backend; make sure jax can
# see it even if the caller pinned JAX_PLATFORMS (e.g. to "cpu").
_jp = os.environ.get("JAX_PLATFORMS")
if _jp is not None and _jp != "" and "axon" not in _jp:
    os.environ["JAX_PLATFORMS"] = "axon," + _jp

M, D, E, H = 65536, 768, 20, 64
N_CORES = 8
MS = M // N_CORES          # rows per core = 8192
GROUP = 512                # rows per PE matmul group
N_GROUPS = MS // GROUP     # 16
SG = 4                     # groups per DMA super-group
N_SG = N_GROUPS // SG      # 4
KCH = D // 128             # 6 contraction chunks
NF = 22                    # 10 dist onehot, 10 cnt onehot, notmask, ones
N_BLK = MS // 128          # 64 feature blocks per core
BIG = float(2 ** 20)       # pre-relu kill value for masked rows

_CACHE = {}


def _build():
    """Build + compile the 8-core SPMD bass program once per process."""
    if "nc" in _CACHE:
        return _CACHE["nc"]

    import concourse.bass as bass
    import concourse.mybir as mybir
    import concourse.tile as tile
    from concourse import bacc
    from concourse.masks import make_identity

    F32 = mybir.dt.float32
    F32R = mybir.dt.float32r

    nc = bacc.Bacc("TRN2", target_bir_lowering=False, debug=False,
                   enable_asserts=False, num_devices=N_CORES)

    xt_d = nc.dram_tensor("xt", [D, MS], F32R, kind="ExternalInput").ap()
    lms_d = nc.dram_tensor("lms", [128, N_BLK], F32, kind="ExternalInput").ap()
    cnt_d = nc.dram_tensor("cnt", [128, N_BLK], F32, kind="ExternalInput").ap()
    w1_d = nc.dram_tensor("w1", [D, H], F32R, kind="ExternalInput").ap()
    tcat_d = nc.dram_tensor("tcat", [NF, H], F32R, kind="ExternalInput").ap()
    wsc_d = nc.dram_tensor("wsc", [H + NF, 1], F32R, kind="ExternalInput").ap()
    lo_d = nc.dram_tensor("lo", [128, NF], F32, kind="ExternalInput").ap()
    hi_d = nc.dram_tensor("hi", [128, NF], F32, kind="ExternalInput").ap()
    out_d = nc.dram_tensor("out", [MS], F32, kind="ExternalOutput").ap()

    # xt[d, m]: tile (k, s) = [128, SG*GROUP] at rows 128k, cols 2048s
    xt_r = xt_d.rearrange("(kp k2 p) (s c) -> p kp k2 s c", p=128, k2=2,
                          s=N_SG)
    w1_r = w1_d.rearrange("(k p) n -> p k n", p=128)    # [128, 6, 64]
    out_r = out_d.rearrange("(s c) -> s c", s=N_SG)  # [4, 2048]

    ge = mybir.AluOpType.is_ge
    le = mybir.AluOpType.is_le
    relu = mybir.ActivationFunctionType.Relu

    with tile.TileContext(nc) as tc:
        with (
            tc.tile_pool(name="consts", bufs=1) as cpool,
            tc.tile_pool(name="feat", bufs=1) as fpool,
            tc.tile_pool(name="xin", bufs=8) as px,
            tc.tile_pool(name="ht", bufs=6) as pht,
            tc.tile_pool(name="osb", bufs=2) as posb,
            tc.tile_pool(name="psf", bufs=2, space="PSUM") as psf,
            tc.tile_pool(name="psz", bufs=4, space="PSUM") as psz,
            tc.tile_pool(name="pss", bufs=2, space="PSUM") as pss,
        ):
            # consts issue on the scalar HWDGE queue so the big xt DMAs
            # (sync queue) start immediately
            ident_t = cpool.tile([128, 128], F32, tag="ident")
            make_identity(nc, ident_t[:])
            ident_r = cpool.tile([128, 128], F32R, tag="identr")
            nc.vector.tensor_copy(ident_r[:], ident_t[:])
            ident = ident_r[:]

            w1t = cpool.tile([128, KCH, H], F32R, tag="w1t")
            nc.scalar.dma_start(w1t[:], w1_r[:])
            lo_t = cpool.tile([128, NF], F32, tag="lo")
            nc.scalar.dma_start(lo_t[:], lo_d[:])
            hi_t = cpool.tile([128, NF], F32, tag="hi")
            nc.scalar.dma_start(hi_t[:], hi_d[:])
            lms_t = cpool.tile([128, N_BLK], F32, tag="lms")
            nc.scalar.dma_start(lms_t[:], lms_d[:])
            cnt_t = cpool.tile([128, N_BLK], F32, tag="cnt")
            nc.scalar.dma_start(cnt_t[:], cnt_d[:])
            tcat_full = cpool.tile([H + NF, H], F32R, tag="tcat")
            tcat = tcat_full[H:H + NF, :]
            nc.scalar.dma_start(tcat, tcat_d[:])
            wsc = cpool.tile([H + NF, 1], F32R, tag="wsc")
            nc.scalar.dma_start(wsc[:], wsc_d[:])

            # F[p, b, i] = onehot / mask features for row m = 128b + p
            tge = fpool.tile([128, N_BLK, NF], F32, tag="tge")
            tle = fpool.tile([128, N_BLK, NF], F32, tag="tle")
            fall = fpool.tile([128, N_BLK, NF], F32R, tag="fall")
            lms_b = lms_t[:, :, None].broadcast_to([128, N_BLK, 10])
            cnt_b = cnt_t[:, :, None].broadcast_to([128, N_BLK, 12])
            nc.vector.tensor_tensor(
                tge[:, :, 0:10], lms_b,
                lo_t[:, None, 0:10].broadcast_to([128, N_BLK, 10]), ge)
            nc.vector.tensor_tensor(
                tge[:, :, 10:NF], cnt_b,
                lo_t[:, None, 10:NF].broadcast_to([128, N_BLK, 12]), ge)
            nc.vector.tensor_tensor(
                tle[:, :, 0:10], lms_b,
                hi_t[:, None, 0:10].broadcast_to([128, N_BLK, 10]), le)
            nc.vector.tensor_tensor(
                tle[:, :, 10:NF], cnt_b,
                hi_t[:, None, 10:NF].broadcast_to([128, N_BLK, 12]), le)
            nc.vector.tensor_mul(fall[:], tge[:], tle[:])

            osb_tiles = {}
            pending = None

            def emit_score(g, ht):
                sc = pss.tile([1, GROUP], F32, tag="pss")
                nc.tensor.matmul(sc[:], wsc[:], ht[:], start=True, stop=True)
                sq = g // SG
                if g % SG == 0:
                    osb_t = posb.tile([1, SG * GROUP], F32, tag="osb")
                    osb_tiles[sq] = osb_t
                orow = osb_tiles[sq][0:1, GROUP * (g % SG):GROUP * (g % SG + 1)]
                if g % 2 == 0:
                    nc.vector.tensor_copy(orow, sc[:])
                else:
                    nc.scalar.copy(orow, sc[:])
                if g % SG == SG - 1:
                    nc.gpsimd.dma_start(out_r[sq:sq + 1, :],
                                        osb_tiles.pop(sq)[:])

            def load_sg(s):
                xts = []
                for kp in range(KCH // 2):
                    xk = px.tile([128, 2, SG * GROUP], F32R, tag="xin")
                    if s == 0:
                        # split so group 0's chunks land first
                        nc.sync.dma_start(xk[:, :, 0:GROUP],
                                          xt_r[:, kp, :, s, 0:GROUP])
                        nc.sync.dma_start(xk[:, :, GROUP:],
                                          xt_r[:, kp, :, s, GROUP:])
                    else:
                        nc.sync.dma_start(xk[:], xt_r[:, kp, :, s, :])
                    xts.append(xk)
                return xts

            sg_tiles = {0: load_sg(0), 1: load_sg(1)}
            for s in range(N_SG):
                if s + 2 < N_SG:
                    sg_tiles[s + 2] = load_sg(s + 2)
                xts = sg_tiles.pop(s)
                for gi in range(SG):
                    g = SG * s + gi
                    off = GROUP * gi
                    if pending is not None:
                        emit_score(*pending)

                    zt = psz.tile([H, GROUP], F32, tag="psz")
                    for k in range(KCH):
                        nc.tensor.matmul(zt[:], w1t[:, k, :],
                                         xts[k // 2][:, k % 2,
                                                     off:off + GROUP],
                                         start=(k == 0), stop=False)

                    # transpose the 4 feature blocks of this group
                    psft = psf.tile([NF, GROUP], F32R, tag="psf")
                    for j in range(4):
                        b = 4 * g + j
                        nc.tensor.transpose(
                            psft[:, 128 * j:128 * (j + 1)],
                            fall[:, b, :], ident)
                    # ht rows 0..63 = relu(z.T), rows 64..85 = F.T
                    ht = pht.tile([H + NF, GROUP], F32R, tag="ht")
                    if g % 2 == 0:
                        nc.vector.tensor_copy(ht[H:H + NF, :], psft[:])
                    else:
                        nc.scalar.copy(ht[H:H + NF, :], psft[:])

                    nc.tensor.matmul(zt[:], tcat, ht[H:H + NF, :],
                                     start=False, stop=True)

                    nc.scalar.activation(ht[0:H, :], zt[:], relu)
                    pending = (g, ht)
                if s == N_SG - 1:
                    emit_score(*pending)
                    pending = None

    nc.compile()
    _CACHE["nc"] = nc
    return nc


def _prepare_maps(ment_emb, mem_vectors, dist_table, counter_table,
                  W1, b1, W2, b2, ent_counter, last_mention_start, ment_start):
    f32 = np.float32
    ment = np.asarray(ment_emb, f32)
    mem = np.asarray(mem_vectors, f32)
    W1 = np.asarray(W1, f32)
    ms = float(np.asarray(ment_start).astype(np.float64))

    W1m, W1r, W1h = W1[0:D], W1[D:2 * D], W1[2 * D:3 * D]
    W1d, W1c = W1[3 * D:3 * D + E], W1[3 * D + E:3 * D + 2 * E]

    w1eff = (W1m + ment[:, None] * W1h).astype(f32)              # [768, 64]
    bias_vec = (np.asarray(b1, f32) + ment @ W1r).astype(f32)    # [64]
    T_d = (np.asarray(dist_table, f32) @ W1d + bias_vec).astype(f32)
    T_c = (np.asarray(counter_table, f32) @ W1c).astype(f32)
    b2v = float(np.asarray(b2, f32).reshape(-1)[0])

    tcat = np.concatenate(
        [T_d, T_c, np.full((1, H), -BIG, f32), np.zeros((1, H), f32)], 0)
    # single score matmul: rows 0..63 act on relu(z.T), rows 64..85 on F.T
    wsc = np.zeros((H + NF, 1), f32)
    wsc[0:H, 0] = np.asarray(W2, f32).reshape(-1)
    wsc[H + 20, 0] = -10000.0 - b2v
    wsc[H + 21, 0] = b2v

    # bucket i covers c in [A[i], B[i]] (identity below 5, log2 above, clip 9)
    A = np.array([-1e9, 1, 2, 3, 4, 5, 8, 16, 32, 64], np.float64)
    B = np.array([0, 1, 2, 3, 4, 7, 15, 31, 63, 1e9], np.float64)
    # dist bucket in lms terms: dist = ms - lms in [A,B] <=> lms in [ms-B, ms-A]
    lo = np.concatenate([ms - B, A, [-1e9], [-1e9]]).astype(f32)
    hi = np.concatenate([ms - A, B, [0.0], [1e9]]).astype(f32)
    lo_rep = np.ascontiguousarray(np.broadcast_to(lo, (128, NF)))
    hi_rep = np.ascontiguousarray(np.broadcast_to(hi, (128, NF)))

    lms_f = np.asarray(last_mention_start).astype(f32)
    cnt_f = np.asarray(ent_counter).astype(f32)

    in_maps = []
    for c in range(N_CORES):
        sl = slice(c * MS, (c + 1) * MS)
        in_maps.append(dict(
            xt=np.ascontiguousarray(mem[sl].T),
            lms=np.ascontiguousarray(lms_f[sl].reshape(N_BLK, 128).T),
            cnt=np.ascontiguousarray(cnt_f[sl].reshape(N_BLK, 128).T),
            w1=w1eff, tcat=tcat, wsc=wsc, lo=lo_rep, hi=hi_rep))
    return in_maps


def _postprocess(results):
    out = np.empty(M + 1, np.float32)
    for c in range(N_CORES):
        out[c * MS:(c + 1) * MS] = results[c]["out"]
    out[M] = 0.0
    return out


def run_spmd(in_maps, trace=False):
    from concourse.bass_utils import run_bass_kernel_spmd
    nc = _build()
    return run_bass_kernel_spmd(nc, in_maps, list(range(N_CORES)), trace=trace)


def kernel(**inputs):
    in_maps = _prepare_maps(**inputs)
    res = run_spmd(in_maps, trace=False)
    return _postprocess(res.results)



# revision 6
# speedup vs baseline: 1.3183x; 1.3183x over previous
"""Trainium2 Bass kernel for the BaseMemory coref scoring module.

Computes, for full inputs (M=65536 memory slots, D=768, E=20, H=64):
    score = relu(pair @ W1 + b1) @ W2 + b2, masked with ent_counter>0,
    where pair = [mem, ment, mem*ment, dist_emb, cnt_emb].

Sharding: data-parallel over the cluster dimension M across 8 NeuronCores.

Device work is a single streamed bf16 matmul pipeline; everything cheap is
folded on the host (O(M) / O(D*H) work):
  - mem@W1_mem + (mem*ment)@W1_had = mem @ (W1_mem + diag(ment)@W1_had)
  - ment@W1_ment + b1 folded into the 10-row dist bucket table
  - bucket embedding lookups precomputed as a [22, M] one-hot/mask matrix,
    contracted on the PE against the folded bucket tables (exact)
  - mem shard pre-cast to bf16 and laid out [group][partition][chunk*col]
    so each group is one fully contiguous 768KB DMA
"""

import os
import numpy as np
from ml_dtypes import bfloat16

_jp = os.environ.get("JAX_PLATFORMS")
if _jp is not None and _jp != "" and "axon" not in _jp:
    os.environ["JAX_PLATFORMS"] = "axon," + _jp

M, D, E, H = 65536, 768, 20, 64
N_CORES = 8
MS = M // N_CORES          # rows per core = 8192
GROUP = 512                # rows per PE matmul group
N_GROUPS = MS // GROUP     # 16
SG = 4                     # groups per output DMA
N_SG = N_GROUPS // SG      # 4
KCH = D // 128             # 6 contraction chunks
NF = 22                    # 10 dist onehot, 10 cnt onehot, masked, ones
BIG = float(2 ** 20)       # pre-relu kill value for masked rows

_CACHE = {}


def _build():
    """Build + compile the 8-core SPMD bass program once per process."""
    if "nc" in _CACHE:
        return _CACHE["nc"]

    import concourse.bass as bass
    import concourse.mybir as mybir
    import concourse.tile as tile
    from concourse import bacc

    F32 = mybir.dt.float32
    BF16 = mybir.dt.bfloat16

    nc = bacc.Bacc("TRN2", target_bir_lowering=False, debug=False,
                   enable_asserts=False, num_devices=N_CORES)

    xt_d = nc.dram_tensor("xt", [N_GROUPS, 128, KCH, GROUP], BF16,
                          kind="ExternalInput").ap()
    f_d = nc.dram_tensor("feat", [NF, MS], BF16, kind="ExternalInput").ap()
    w1_d = nc.dram_tensor("w1", [128, KCH, H], BF16,
                          kind="ExternalInput").ap()
    tcat_d = nc.dram_tensor("tcat", [NF, H], BF16, kind="ExternalInput").ap()
    wsc_d = nc.dram_tensor("wsc", [H + 2, 1], BF16,
                           kind="ExternalInput").ap()
    out_d = nc.dram_tensor("out", [MS], F32, kind="ExternalOutput").ap()

    f_r = f_d.rearrange("f (g c) -> f g c", g=N_GROUPS)
    out_r = out_d.rearrange("(s c) -> s c", s=N_SG)  # [4, 2048]

    relu = mybir.ActivationFunctionType.Relu

    with tile.TileContext(nc) as tc:
        with (
            tc.tile_pool(name="consts", bufs=1) as cpool,
            tc.tile_pool(name="xin", bufs=N_GROUPS) as px,
            tc.tile_pool(name="ht", bufs=4) as pht,
            tc.tile_pool(name="osb", bufs=2) as posb,
            tc.tile_pool(name="psz", bufs=4, space="PSUM") as psz,
            tc.tile_pool(name="pss", bufs=2, space="PSUM") as pss,
        ):
            # consts go on the scalar HWDGE queue so the big xt DMAs
            # (sync queue) start immediately
            w1t = cpool.tile([128, KCH, H], BF16, tag="w1t")
            nc.scalar.dma_start(w1t[:], w1_d[:])
            tcat = cpool.tile([NF, H], BF16, tag="tcat")
            nc.scalar.dma_start(tcat[:], tcat_d[:])
            wsc = cpool.tile([H + 2, 1], BF16, tag="wsc")
            nc.scalar.dma_start(wsc[:], wsc_d[:])
            fall = cpool.tile([NF, N_GROUPS, GROUP], BF16, tag="fall")
            nc.scalar.dma_start(fall[:], f_r[:])

            # stream the whole shard: 16 × 768KB contiguous reads,
            # issued upfront so the sync queue never starves
            xts = []
            for g in range(N_GROUPS):
                xg = px.tile([128, KCH, GROUP], BF16, tag="xin")
                nc.sync.dma_start(xg[:], xt_d[g])
                xts.append(xg)

            osb_tiles = {}
            pending = None

            def emit_score(g, ht):
                sc = pss.tile([1, GROUP], F32, tag="pss")
                nc.tensor.matmul(sc[:], wsc[:], ht[:], start=True, stop=True)
                sq, r = divmod(g, SG)
                if r == 0:
                    osb_t = posb.tile([1, SG * GROUP], F32, tag="osb")
                    osb_tiles[sq] = osb_t
                orow = osb_tiles[sq][0:1, GROUP * r:GROUP * (r + 1)]
                if g % 2 == 0:
                    nc.vector.tensor_copy(orow, sc[:])
                else:
                    nc.scalar.copy(orow, sc[:])
                if r == SG - 1:
                    nc.gpsimd.dma_start(out_r[sq:sq + 1, :],
                                        osb_tiles.pop(sq)[:])

            for g in range(N_GROUPS):
                if pending is not None:
                    emit_score(*pending)
                zt = psz.tile([H, GROUP], F32, tag="psz")
                xg = xts[g]
                for k in range(KCH):
                    nc.tensor.matmul(zt[:], w1t[:, k, :], xg[:, k, :],
                                     start=(k == 0), stop=False)
                nc.tensor.matmul(zt[:], tcat[:], fall[:, g, :],
                                 start=False, stop=True)
                ht = pht.tile([H + 2, GROUP], BF16, tag="ht")
                nc.scalar.activation(ht[0:H, :], zt[:], relu)
                nc.vector.tensor_copy(ht[H:H + 2, :], fall[0:2, g, :])
                pending = (g, ht)
            emit_score(*pending)

    nc.compile()
    _CACHE["nc"] = nc
    return nc


def _get_bucket(c):
    """Identity buckets for c<=4, floor(log2) buckets above, clamped to
    [0, 9]. Integer-exact; matches the f32 jax reference on [0, 2^20]."""
    c = np.asarray(c).astype(np.int64)
    cl = np.maximum(c, 1)
    fl = np.frexp(cl.astype(np.float64))[1] - 1   # floor(log2), exact
    idx = np.where(c <= 4, c, fl + 3)
    return np.clip(idx, 0, 9).astype(np.int64)


def _prepare_maps(ment_emb, mem_vectors, dist_table, counter_table,
                  W1, b1, W2, b2, ent_counter, last_mention_start, ment_start):
    f32 = np.float32
    ment = np.asarray(ment_emb, f32)
    mem = np.asarray(mem_vectors, f32)
    W1 = np.asarray(W1, f32)
    ms_i = int(np.asarray(ment_start))

    W1m, W1r, W1h = W1[0:D], W1[D:2 * D], W1[2 * D:3 * D]
    W1d, W1c = W1[3 * D:3 * D + E], W1[3 * D + E:3 * D + 2 * E]

    w1eff = (W1m + ment[:, None] * W1h).astype(f32)              # [768, 64]
    bias_vec = (np.asarray(b1, f32) + ment @ W1r).astype(f32)    # [64]
    T_d = (np.asarray(dist_table, f32) @ W1d + bias_vec).astype(f32)
    T_c = (np.asarray(counter_table, f32) @ W1c).astype(f32)
    b2v = float(np.asarray(b2, f32).reshape(-1)[0])

    # feature row order: [masked, ones, dist onehots, cnt onehots] — the
    # masked/ones rows sit at partitions 0:2 so the ht copy is 32-aligned
    tcat = np.concatenate(
        [np.full((1, H), -BIG, f32), np.zeros((1, H), f32), T_d, T_c],
        0).astype(bfloat16)
    wsc = np.zeros((H + 2, 1), f32)
    wsc[0:H, 0] = np.asarray(W2, f32).reshape(-1)
    wsc[H, 0] = -10000.0 - b2v       # masked-row score
    wsc[H + 1, 0] = b2v              # layer-2 bias via the ones row
    wsc = wsc.astype(bfloat16)

    w1t = np.ascontiguousarray(
        w1eff.reshape(KCH, 128, H).transpose(1, 0, 2)).astype(bfloat16)

    cnt = np.asarray(ent_counter).astype(np.int64)
    lms = np.asarray(last_mention_start).astype(np.int64)
    bd = _get_bucket(ms_i - lms)
    bc = _get_bucket(cnt)
    F = np.zeros((NF, M), f32)
    rows = np.arange(M)
    F[0] = (cnt <= 0)
    F[1] = 1.0
    F[2 + bd, rows] = 1.0
    F[12 + bc, rows] = 1.0
    F = F.astype(bfloat16)

    mem16 = mem.astype(bfloat16)

    in_maps = []
    for c in range(N_CORES):
        sl = slice(c * MS, (c + 1) * MS)
        xt = np.ascontiguousarray(
            mem16[sl].reshape(N_GROUPS, GROUP, KCH, 128).transpose(0, 3, 2, 1))
        in_maps.append(dict(
            xt=xt, feat=np.ascontiguousarray(F[:, sl]),
            w1=w1t, tcat=tcat, wsc=wsc))
    return in_maps


def _postprocess(results):
    out = np.empty(M + 1, np.float32)
    for c in range(N_CORES):
        out[c * MS:(c + 1) * MS] = results[c]["out"]
    out[M] = 0.0
    return out


def run_spmd(in_maps, trace=False):
    from concourse.bass_utils import run_bass_kernel_spmd
    nc = _build()
    return run_bass_kernel_spmd(nc, in_maps, list(range(N_CORES)), trace=trace)


def kernel(**inputs):
    in_maps = _prepare_maps(**inputs)
    res = run_spmd(in_maps, trace=False)
    return _postprocess(res.results)


# revision 7
# speedup vs baseline: 1.6230x; 1.2311x over previous
"""Trainium2 Bass kernel for the BaseMemory coref scoring module.

Computes, for full inputs (M=65536 memory slots, D=768, E=20, H=64):
    score = relu(pair @ W1 + b1) @ W2 + b2, masked with ent_counter>0,
    where pair = [mem, ment, mem*ment, dist_emb, cnt_emb].

Sharding: data-parallel over the cluster dimension M across 8 NeuronCores.

Device work is a single streamed bf16 matmul pipeline; everything cheap is
folded on the host (O(M) / O(D*H) work):
  - mem@W1_mem + (mem*ment)@W1_had = mem @ (W1_mem + diag(ment)@W1_had)
  - ment@W1_ment + b1 folded into the 10-row dist bucket table
  - bucket embedding lookups precomputed as a [21, M] one-hot/mask matrix,
    contracted on the PE against the folded bucket tables (exact)
  - mem shard pre-cast to bf16, laid out so each DMA is one contiguous 1MB
    read delivering one chunk-pair across 4 row-groups
  - PE schedule batches 4 same-weight matmuls per LDWEIGHTS; scores for two
    groups share one matmul via a block-diagonal W2; the mask/bias terms are
    a host-precomputed row added by the vector engine on the way out
"""

import os
import numpy as np
from ml_dtypes import bfloat16

_jp = os.environ.get("JAX_PLATFORMS")
if _jp is not None and _jp != "" and "axon" not in _jp:
    os.environ["JAX_PLATFORMS"] = "axon," + _jp

M, D, E, H = 65536, 768, 20, 64
N_CORES = 8
MS = M // N_CORES          # rows per core = 8192
GROUP = 512                # rows per PE matmul
N_GROUPS = MS // GROUP     # 16
SG = 4                     # groups per supergroup (weight-batch unit)
N_SG = N_GROUPS // SG      # 4
KCH = D // 128             # 6 contraction chunks
KP = KCH // 2              # 3 chunk-pairs (1MB DMA units)
NF = 21                    # masked flag + 10 dist onehot + 10 cnt onehot
BIG = float(2 ** 20)       # pre-relu kill value for masked rows

_CACHE = {}


def _build():
    """Build + compile the 8-core SPMD bass program once per process."""
    if "nc" in _CACHE:
        return _CACHE["nc"]

    import concourse.bass as bass
    import concourse.mybir as mybir
    import concourse.tile as tile
    from concourse import bacc

    F32 = mybir.dt.float32
    BF16 = mybir.dt.bfloat16
    add = mybir.AluOpType.add

    nc = bacc.Bacc("TRN2", target_bir_lowering=False, debug=False,
                   enable_asserts=False, num_devices=N_CORES)

    xt_d = nc.dram_tensor("xt", [N_SG, KP, 128, 2, SG * GROUP], BF16,
                          kind="ExternalInput").ap()
    f_d = nc.dram_tensor("feat", [NF, MS], BF16, kind="ExternalInput").ap()
    w1_d = nc.dram_tensor("w1", [128, KCH, H], BF16,
                          kind="ExternalInput").ap()
    tcat_d = nc.dram_tensor("tcat", [NF, H], BF16, kind="ExternalInput").ap()
    wsc_d = nc.dram_tensor("wsc", [128, 2], BF16, kind="ExternalInput").ap()
    fc_d = nc.dram_tensor("fc", [2, N_SG, 2, GROUP], F32,
                          kind="ExternalInput").ap()
    out_d = nc.dram_tensor("out", [MS], F32, kind="ExternalOutput").ap()

    f_r = f_d.rearrange("f (g c) -> f g c", g=N_GROUPS)
    # m = 2048*s + 1024*jj + 512*r + c  ->  [s][r, jj, c]
    out_r = out_d.rearrange("(s jj r c) -> s r jj c", jj=2, r=2, c=GROUP)

    relu = mybir.ActivationFunctionType.Relu

    with tile.TileContext(nc) as tc:
        with (
            tc.tile_pool(name="consts", bufs=1) as cpool,
            tc.tile_pool(name="xin", bufs=N_SG * KP) as px,
            tc.tile_pool(name="ht", bufs=4) as pht,
            tc.tile_pool(name="osb", bufs=2) as posb,
            tc.tile_pool(name="psz", bufs=6, space="PSUM") as psz,
            tc.tile_pool(name="pss", bufs=2, space="PSUM") as pss,
        ):
            # consts go on the scalar HWDGE queue so the big xt DMAs
            # (sync queue) start immediately
            w1t = cpool.tile([128, KCH, H], BF16, tag="w1t")
            nc.scalar.dma_start(w1t[:], w1_d[:])
            tcat = cpool.tile([NF, H], BF16, tag="tcat")
            nc.scalar.dma_start(tcat[:], tcat_d[:])
            wsc2 = cpool.tile([128, 2], BF16, tag="wsc2")
            nc.scalar.dma_start(wsc2[:], wsc_d[:])
            fall = cpool.tile([NF, N_GROUPS, GROUP], BF16, tag="fall")
            nc.scalar.dma_start(fall[:], f_r[:])
            fct = cpool.tile([2, N_SG, 2, GROUP], F32, tag="fct")
            nc.scalar.dma_start(fct[:], fc_d[:])

            # stream the whole shard: 12 x 1MB contiguous reads, issued
            # upfront so the sync queue never starves
            xts = []
            for s in range(N_SG):
                row = []
                for kp in range(KP):
                    xk = px.tile([128, 2, SG * GROUP], BF16, tag="xin")
                    nc.sync.dma_start(xk[:], xt_d[s, kp])
                    row.append(xk)
                xts.append(row)

            def emit_scores(s, hts):
                osb2 = posb.tile([2, 2, GROUP], F32, tag="osb")
                for jj in range(2):
                    sc2 = pss.tile([2, GROUP], F32, tag="pss")
                    nc.tensor.matmul(sc2[:], wsc2[:], hts[jj][:],
                                     start=True, stop=True)
                    nc.vector.tensor_tensor(osb2[:, jj, :], sc2[:],
                                            fct[:, s, jj, :], add)
                nc.gpsimd.dma_start(out_r[s], osb2[:])

            pending = None
            for s in range(N_SG):
                zts = []
                for j in range(SG):
                    zt = psz.tile([H, GROUP], F32, tag="psz")
                    zts.append(zt)
                for kp in range(KP):
                    xk = xts[s][kp]
                    for kk in range(2):
                        k = 2 * kp + kk
                        for j in range(SG):
                            nc.tensor.matmul(
                                zts[j][:], w1t[:, k, :],
                                xk[:, kk, GROUP * j:GROUP * (j + 1)],
                                start=(k == 0), stop=False)
                for j in range(SG):
                    nc.tensor.matmul(zts[j][:], tcat[:],
                                     fall[:, SG * s + j, :],
                                     start=False, stop=True)
                # previous supergroup's scores: relus had a full main pass
                if pending is not None:
                    emit_scores(*pending)
                hts = []
                for jj in range(2):
                    htp = pht.tile([128, GROUP], BF16, tag="ht")
                    hts.append(htp)
                for j in range(SG):
                    dst = hts[j // 2][H * (j % 2):H * (j % 2 + 1), :]
                    if j % 2 == 0:
                        nc.scalar.activation(dst, zts[j][:], relu)
                    else:
                        nc.vector.tensor_scalar_max(dst, zts[j][:], 0.0)
                pending = (s, hts)
            emit_scores(*pending)

    nc.compile()
    _CACHE["nc"] = nc
    return nc


def _get_bucket(c):
    """Identity buckets for c<=4, floor(log2) buckets above, clamped to
    [0, 9]. Integer-exact; matches the f32 jax reference on [0, 2^20]."""
    c = np.asarray(c).astype(np.int64)
    cl = np.maximum(c, 1)
    fl = np.frexp(cl.astype(np.float64))[1] - 1   # floor(log2), exact
    idx = np.where(c <= 4, c, fl + 3)
    return np.clip(idx, 0, 9).astype(np.int64)


def _prepare_maps(ment_emb, mem_vectors, dist_table, counter_table,
                  W1, b1, W2, b2, ent_counter, last_mention_start, ment_start):
    f32 = np.float32
    ment = np.asarray(ment_emb, f32)
    mem = np.asarray(mem_vectors, f32)
    W1 = np.asarray(W1, f32)
    ms_i = int(np.asarray(ment_start))

    W1m, W1r, W1h = W1[0:D], W1[D:2 * D], W1[2 * D:3 * D]
    W1d, W1c = W1[3 * D:3 * D + E], W1[3 * D + E:3 * D + 2 * E]

    w1eff = (W1m + ment[:, None] * W1h).astype(f32)              # [768, 64]
    bias_vec = (np.asarray(b1, f32) + ment @ W1r).astype(f32)    # [64]
    T_d = (np.asarray(dist_table, f32) @ W1d + bias_vec).astype(f32)
    T_c = (np.asarray(counter_table, f32) @ W1c).astype(f32)
    b2v = float(np.asarray(b2, f32).reshape(-1)[0])

    # feature rows: [masked, dist onehots, cnt onehots]; masked row kills
    # z pre-relu so masked scores come exactly from the fc row below
    tcat = np.concatenate(
        [np.full((1, H), -BIG, f32), T_d, T_c], 0).astype(bfloat16)
    wsc2 = np.zeros((128, 2), f32)
    wsc2[0:H, 0] = np.asarray(W2, f32).reshape(-1)
    wsc2[H:2 * H, 1] = np.asarray(W2, f32).reshape(-1)
    wsc2 = wsc2.astype(bfloat16)

    w1t = np.ascontiguousarray(
        w1eff.reshape(KCH, 128, H).transpose(1, 0, 2)).astype(bfloat16)

    cnt = np.asarray(ent_counter).astype(np.int64)
    lms = np.asarray(last_mention_start).astype(np.int64)
    bd = _get_bucket(ms_i - lms)
    bc = _get_bucket(cnt)
    F = np.zeros((NF, M), f32)
    rows = np.arange(M)
    masked = (cnt <= 0)
    F[0] = masked
    F[1 + bd, rows] = 1.0
    F[11 + bc, rows] = 1.0
    F = F.astype(bfloat16)

    # score = W2.relu(z) + fc,  fc = masked*(-10000-b2) + b2  (exact masking)
    fcrow = np.where(masked, np.float32(-10000.0), np.float32(b2v))

    mem16 = mem.astype(bfloat16)

    in_maps = []
    for c in range(N_CORES):
        sl = slice(c * MS, (c + 1) * MS)
        # [m, d] -> [s, kp, p, kk, j, c]; m = 2048s+512j+c, d = 256kp+128kk+p
        xt = np.ascontiguousarray(
            mem16[sl].reshape(N_SG, SG, GROUP, KP, 2, 128)
            .transpose(0, 3, 5, 4, 1, 2).reshape(N_SG, KP, 128, 2, SG * GROUP))
        fc = np.ascontiguousarray(
            fcrow[sl].reshape(N_SG, 2, 2, GROUP).transpose(2, 0, 1, 3))
        in_maps.append(dict(
            xt=xt, feat=np.ascontiguousarray(F[:, sl]),
            w1=w1t, tcat=tcat, wsc=wsc2, fc=fc))
    return in_maps


def _postprocess(results):
    out = np.empty(M + 1, np.float32)
    for c in range(N_CORES):
        out[c * MS:(c + 1) * MS] = results[c]["out"]
    out[M] = 0.0
    return out


def run_spmd(in_maps, trace=False):
    from concourse.bass_utils import run_bass_kernel_spmd
    nc = _build()
    return run_bass_kernel_spmd(nc, in_maps, list(range(N_CORES)), trace=trace)


def kernel(**inputs):
    in_maps = _prepare_maps(**inputs)
    res = run_spmd(in_maps, trace=False)
    return _postprocess(res.results)


# revision 9
# speedup vs baseline: 1.6582x; 1.0217x over previous
"""Trainium2 Bass kernel for the BaseMemory coref scoring module.

Computes, for full inputs (M=65536 memory slots, D=768, E=20, H=64):
    score = relu(pair @ W1 + b1) @ W2 + b2, masked with ent_counter>0,
    where pair = [mem, ment, mem*ment, dist_emb, cnt_emb].

Sharding: data-parallel over the cluster dimension M across 8 NeuronCores.

Device work is a single streamed bf16 matmul pipeline; everything cheap is
folded on the host (O(M) / O(D*H) work):
  - mem@W1_mem + (mem*ment)@W1_had = mem @ (W1_mem + diag(ment)@W1_had)
  - ment@W1_ment + b1 folded into the 10-row dist bucket table
  - bucket embedding lookups precomputed as a [21, M] one-hot/mask matrix,
    contracted on the PE against the folded bucket tables (exact)
  - mem shard pre-cast to bf16, laid out so each DMA is one contiguous 1MB
    read delivering one chunk-pair across 4 row-groups
  - PE schedule batches 4 same-weight matmuls per LDWEIGHTS; scores for two
    groups share one matmul via a block-diagonal W2; the mask/bias terms are
    a host-precomputed row added by the vector engine on the way out
"""

import os
import numpy as np
from ml_dtypes import bfloat16

_jp = os.environ.get("JAX_PLATFORMS")
if _jp is not None and _jp != "" and "axon" not in _jp:
    os.environ["JAX_PLATFORMS"] = "axon," + _jp

M, D, E, H = 65536, 768, 20, 64
N_CORES = 8
MS = M // N_CORES          # rows per core = 8192
GROUP = 512                # rows per PE matmul
N_GROUPS = MS // GROUP     # 16
SG = 4                     # groups per supergroup (weight-batch unit)
N_SG = N_GROUPS // SG      # 4
KCH = D // 128             # 6 contraction chunks
KP = KCH // 2              # 3 chunk-pairs (1MB DMA units)
NF = 21                    # masked flag + 10 dist onehot + 10 cnt onehot
BIG = float(2 ** 20)       # pre-relu kill value for masked rows

_CACHE = {}


def _build():
    """Build + compile the 8-core SPMD bass program once per process."""
    if "nc" in _CACHE:
        return _CACHE["nc"]

    import concourse.bass as bass
    import concourse.mybir as mybir
    import concourse.tile as tile
    from concourse import bacc

    F32 = mybir.dt.float32
    BF16 = mybir.dt.bfloat16
    add = mybir.AluOpType.add

    nc = bacc.Bacc("TRN2", target_bir_lowering=False, debug=False,
                   enable_asserts=False, num_devices=N_CORES)

    xt_d = nc.dram_tensor("xt", [N_SG, KP, 128, 2, SG * GROUP], BF16,
                          kind="ExternalInput").ap()
    f_d = nc.dram_tensor("feat", [NF, MS], BF16, kind="ExternalInput").ap()
    w1_d = nc.dram_tensor("w1", [128, KCH, H], BF16,
                          kind="ExternalInput").ap()
    tcat_d = nc.dram_tensor("tcat", [NF, H], BF16, kind="ExternalInput").ap()
    wsc_d = nc.dram_tensor("wsc", [128, 2], BF16, kind="ExternalInput").ap()
    fc_d = nc.dram_tensor("fc", [2, N_SG, 2, GROUP], F32,
                          kind="ExternalInput").ap()
    out_d = nc.dram_tensor("out", [MS], F32, kind="ExternalOutput").ap()

    f_r = f_d.rearrange("f (g c) -> f g c", g=N_GROUPS)
    # m = 2048*s + 1024*jj + 512*r + c  ->  [s][r, jj, c]
    out_r = out_d.rearrange("(s jj r c) -> s r jj c", jj=2, r=2, c=GROUP)

    relu = mybir.ActivationFunctionType.Relu

    with tile.TileContext(nc) as tc:
        with (
            tc.tile_pool(name="consts", bufs=1) as cpool,
            tc.tile_pool(name="xin", bufs=N_SG * KP) as px,
            tc.tile_pool(name="ht", bufs=4) as pht,
            tc.tile_pool(name="osb", bufs=2) as posb,
            tc.tile_pool(name="psz", bufs=6, space="PSUM") as psz,
            tc.tile_pool(name="pss", bufs=2, space="PSUM") as pss,
        ):
            # consts go on the scalar HWDGE queue so the big xt DMAs
            # (sync queue) start immediately
            w1t = cpool.tile([128, KCH, H], BF16, tag="w1t")
            nc.scalar.dma_start(w1t[:], w1_d[:])
            tcat = cpool.tile([NF, H], BF16, tag="tcat")
            nc.scalar.dma_start(tcat[:], tcat_d[:])
            wsc2 = cpool.tile([128, 2], BF16, tag="wsc2")
            nc.scalar.dma_start(wsc2[:], wsc_d[:])
            fall = cpool.tile([NF, N_GROUPS, GROUP], BF16, tag="fall")
            nc.scalar.dma_start(fall[:], f_r[:])
            fct = cpool.tile([2, N_SG, 2, GROUP], F32, tag="fct")
            nc.scalar.dma_start(fct[:], fc_d[:])

            # stream the whole shard: 1MB contiguous reads, issued upfront so
            # the sync queue never starves; the last supergroup is split into
            # per-chunk 512KB slices so little work remains after final byte
            xts = []
            for s in range(N_SG):
                row = []
                for kp in range(KP):
                    xk = px.tile([128, 2, SG * GROUP], BF16, tag="xin")
                    if s == N_SG - 1:
                        nc.sync.dma_start(xk[:, 0, :], xt_d[s, kp, :, 0, :])
                        nc.sync.dma_start(xk[:, 1, :], xt_d[s, kp, :, 1, :])
                    else:
                        nc.sync.dma_start(xk[:], xt_d[s, kp])
                    row.append(xk)
                xts.append(row)

            def emit_scores(s, hts):
                osb2 = posb.tile([2, 2, GROUP], F32, tag="osb")
                for jj in range(2):
                    sc2 = pss.tile([2, GROUP], F32, tag="pss")
                    nc.tensor.matmul(sc2[:], wsc2[:], hts[jj][:],
                                     start=True, stop=True)
                    nc.vector.tensor_tensor(osb2[:, jj, :], sc2[:],
                                            fct[:, s, jj, :], add)
                # HWDGE out: the sync ring is idle once the input stream ends,
                # and avoids the ~1µs SWDGE fixed cost on the critical tail
                nc.sync.dma_start(out_r[s], osb2[:])

            pending = None
            for s in range(N_SG):
                zts = []
                for j in range(SG):
                    zt = psz.tile([H, GROUP], F32, tag="psz")
                    zts.append(zt)
                for kp in range(KP):
                    xk = xts[s][kp]
                    for kk in range(2):
                        k = 2 * kp + kk
                        for j in range(SG):
                            nc.tensor.matmul(
                                zts[j][:], w1t[:, k, :],
                                xk[:, kk, GROUP * j:GROUP * (j + 1)],
                                start=(k == 0), stop=False)
                for j in range(SG):
                    nc.tensor.matmul(zts[j][:], tcat[:],
                                     fall[:, SG * s + j, :],
                                     start=False, stop=True)
                # previous supergroup's scores: relus had a full main pass
                if pending is not None:
                    emit_scores(*pending)
                hts = []
                for jj in range(2):
                    htp = pht.tile([128, GROUP], BF16, tag="ht")
                    hts.append(htp)
                for j in range(SG):
                    dst = hts[j // 2][H * (j % 2):H * (j % 2 + 1), :]
                    if j % 2 == 0:
                        nc.scalar.activation(dst, zts[j][:], relu)
                    else:
                        nc.vector.tensor_scalar_max(dst, zts[j][:], 0.0)
                pending = (s, hts)
            emit_scores(*pending)

    nc.compile()
    _CACHE["nc"] = nc
    return nc


def _get_bucket(c):
    """Identity buckets for c<=4, floor(log2) buckets above, clamped to
    [0, 9]. Integer-exact; matches the f32 jax reference on [0, 2^20]."""
    c = np.asarray(c).astype(np.int64)
    cl = np.maximum(c, 1)
    fl = np.frexp(cl.astype(np.float64))[1] - 1   # floor(log2), exact
    idx = np.where(c <= 4, c, fl + 3)
    return np.clip(idx, 0, 9).astype(np.int64)


def _prepare_maps(ment_emb, mem_vectors, dist_table, counter_table,
                  W1, b1, W2, b2, ent_counter, last_mention_start, ment_start):
    f32 = np.float32
    ment = np.asarray(ment_emb, f32)
    mem = np.asarray(mem_vectors, f32)
    W1 = np.asarray(W1, f32)
    ms_i = int(np.asarray(ment_start))

    W1m, W1r, W1h = W1[0:D], W1[D:2 * D], W1[2 * D:3 * D]
    W1d, W1c = W1[3 * D:3 * D + E], W1[3 * D + E:3 * D + 2 * E]

    w1eff = (W1m + ment[:, None] * W1h).astype(f32)              # [768, 64]
    bias_vec = (np.asarray(b1, f32) + ment @ W1r).astype(f32)    # [64]
    T_d = (np.asarray(dist_table, f32) @ W1d + bias_vec).astype(f32)
    T_c = (np.asarray(counter_table, f32) @ W1c).astype(f32)
    b2v = float(np.asarray(b2, f32).reshape(-1)[0])

    # feature rows: [masked, dist onehots, cnt onehots]; masked row kills
    # z pre-relu so masked scores come exactly from the fc row below
    tcat = np.concatenate(
        [np.full((1, H), -BIG, f32), T_d, T_c], 0).astype(bfloat16)
    wsc2 = np.zeros((128, 2), f32)
    wsc2[0:H, 0] = np.asarray(W2, f32).reshape(-1)
    wsc2[H:2 * H, 1] = np.asarray(W2, f32).reshape(-1)
    wsc2 = wsc2.astype(bfloat16)

    w1t = np.ascontiguousarray(
        w1eff.reshape(KCH, 128, H).transpose(1, 0, 2)).astype(bfloat16)

    cnt = np.asarray(ent_counter).astype(np.int64)
    lms = np.asarray(last_mention_start).astype(np.int64)
    bd = _get_bucket(ms_i - lms)
    bc = _get_bucket(cnt)
    F = np.zeros((NF, M), f32)
    rows = np.arange(M)
    masked = (cnt <= 0)
    F[0] = masked
    F[1 + bd, rows] = 1.0
    F[11 + bc, rows] = 1.0
    F = F.astype(bfloat16)

    # score = W2.relu(z) + fc,  fc = masked*(-10000-b2) + b2  (exact masking)
    fcrow = np.where(masked, np.float32(-10000.0), np.float32(b2v))

    mem16 = mem.astype(bfloat16)

    in_maps = []
    for c in range(N_CORES):
        sl = slice(c * MS, (c + 1) * MS)
        # [m, d] -> [s, kp, p, kk, j, c]; m = 2048s+512j+c, d = 256kp+128kk+p
        xt = np.ascontiguousarray(
            mem16[sl].reshape(N_SG, SG, GROUP, KP, 2, 128)
            .transpose(0, 3, 5, 4, 1, 2).reshape(N_SG, KP, 128, 2, SG * GROUP))
        fc = np.ascontiguousarray(
            fcrow[sl].reshape(N_SG, 2, 2, GROUP).transpose(2, 0, 1, 3))
        in_maps.append(dict(
            xt=xt, feat=np.ascontiguousarray(F[:, sl]),
            w1=w1t, tcat=tcat, wsc=wsc2, fc=fc))
    return in_maps


def _postprocess(results):
    out = np.empty(M + 1, np.float32)
    for c in range(N_CORES):
        out[c * MS:(c + 1) * MS] = results[c]["out"]
    out[M] = 0.0
    return out


def run_spmd(in_maps, trace=False):
    from concourse.bass_utils import run_bass_kernel_spmd
    nc = _build()
    return run_bass_kernel_spmd(nc, in_maps, list(range(N_CORES)), trace=trace)


def kernel(**inputs):
    in_maps = _prepare_maps(**inputs)
    res = run_spmd(in_maps, trace=False)
    return _postprocess(res.results)
